# revision 11
# baseline (speedup 1.0000x reference)
"""Trainium2 Bass kernel for nn_BinaryPathEncoder (v2).

Math: for each position p, R(p) = M_{b0} @ M_{b1} @ ... (LSB-first binary
path, leading 1 stripped), M_b = expm(B_b - B_b^T)^T.  Split the <=16-step
path 6+5+5 so R(p) = R_A @ R_B @ R_C with table lookups:
  mm1: X1T = (R_A R_B)^T = matmul(lhsT=Rn[idxB] (staged), rhs=Rt[idxA])
  mm2: O   = R_A R_B R_C  = matmul(lhsT=X1T (staged),     rhs=Rn[idxC])
Tables are f16 in SBUF; expm is computed on device (f32r Taylor-Horner
with scaled-A trick, S=2/N=5, two transpose-free chains per primitive).

v2 vs v1 (577us):
- Data-dependent moving-operand offsets come from batched multi-register
  TensorLoads (48 regs / 16 positions) used directly as AP offsets --
  no per-pair reg ALU/snap traffic on the PE queue.
- Transposed table stored as two kc-plane tensors (rt0/rt1) so ONE
  register serves both K-passes of mm1.
- Output DMAs batched 4 positions per dma_start (64 instead of 512).
- GpSimd stag_b gather offsets loaded 8 at a time.
- expm: 4 chains (b x {E, E^T}) interleaved on the PE, adds on DVE with
  pre-scaled A/k buffers (one tensor_tensor per Horner step), final
  squaring skipped for the E^T chains.
"""

import contextlib
import numpy as np

DIM = 256
NCORES = 8
P = 128

NAT_E = 63                     # natural table entries (q in [1,64))
TRA_E = 65                     # transposed: slot0=identity, 1..64 = q in [64,128)
ENT = 512                      # natural entry elements per partition (2 kc x 256)
NSTRIDE = NAT_E * ENT          # per-partition stride of rn_all
TSTRIDE = TRA_E * DIM          # per-partition stride of rt0/rt1

BATCH = 16                     # PE positions per register batch (48 regs)
GB = 8                         # gpsimd staging batch (8 regs)
NSB = 16                       # stag_b ring slots
NSX = 16                       # stag_x ring slots
NOUT = 8                       # outb slots (2 dma groups of 4)
EXPM_S = 2                     # A = skew / 2^S
EXPM_N = 5                     # Taylor order

_NC_CACHE = {}
LAST_RESULTS = None


def _build_nc(npos, debug=False, dump=False):
    from concourse import bass, bacc, mybir

    f32 = mybir.dt.float32
    f32r = mybir.dt.float32r
    f16 = mybir.dt.float16
    i32 = mybir.dt.int32
    Sub = mybir.AluOpType.subtract
    Add = mybir.AluOpType.add
    Eq = mybir.AluOpType.is_equal

    assert npos % BATCH == 0
    NB = npos // BATCH
    NGB = npos // GB
    NGRP = npos // 4

    nc = bacc.Bacc("TRN2", target_bir_lowering=False, debug=debug)

    prims_ext = nc.dram_tensor("prims", [2, DIM, DIM], f32, kind="ExternalInput")
    primsT_ext = nc.dram_tensor("primsT", [2, DIM, DIM], f32, kind="ExternalInput")
    offs_ac_ext = nc.dram_tensor("offs_ac", [P, 48], i32, kind="ExternalInput")
    offs_gp_ext = nc.dram_tensor("offs_gp", [P, GB], i32, kind="ExternalInput")
    out_ext = nc.dram_tensor("out", [npos, 2, P, DIM], f16, kind="ExternalOutput")
    if dump:
        dbg_rn = nc.dram_tensor("dbg_rn", [P, NAT_E * 2 * DIM], f16,
                                kind="ExternalOutput")
        dbg_rt0 = nc.dram_tensor("dbg_rt0", [P, TRA_E * DIM], f16,
                                 kind="ExternalOutput")
        dbg_rt1 = nc.dram_tensor("dbg_rt1", [P, TRA_E * DIM], f16,
                                 kind="ExternalOutput")
        dbg_pbf = nc.dram_tensor("dbg_pbf", [P, 2 * 2 * DIM], f16,
                                 kind="ExternalOutput")

    inv2s = 1.0 / (2.0 ** EXPM_S)

    with contextlib.ExitStack() as ctx:
        sem = {}
        for name in ["in_sem", "gps_sem", "pe_sem", "dve_sem", "act_sem",
                     "mm1_sem", "mm2_sem", "dvex_sem", "actp_sem", "stg_sem",
                     "dma_s0", "dma_s1"]:
            sem[name] = ctx.enter_context(nc.semaphore(name))

        # ---- persistent SBUF ----
        rt0 = ctx.enter_context(nc.sbuf_tensor("rt0", [P, TSTRIDE], f16))
        rt1 = ctx.enter_context(nc.sbuf_tensor("rt1", [P, TSTRIDE], f16))
        rn_all = ctx.enter_context(nc.sbuf_tensor("rn_all", [P, NAT_E, 2, DIM], f16))
        pbf = ctx.enter_context(nc.sbuf_tensor("pbf", [P, 2, 2, DIM], f16))
        offs_ac = ctx.enter_context(nc.sbuf_tensor("offs_ac_sb", [P, 48], i32))
        offs_gp = ctx.enter_context(nc.sbuf_tensor("offs_gp_sb", [P, GB], i32))

        # ---- expm-phase buffers; SBUF space reused by the position-phase
        # staging buffers (disjoint runtime lifetimes, ordered by the build
        # barrier) ----
        tmp_ctx = contextlib.ExitStack()
        prim = [tmp_ctx.enter_context(nc.sbuf_tensor(f"prim{b}", [P, 2, DIM], f32))
                for b in range(2)]
        primT = [tmp_ctx.enter_context(nc.sbuf_tensor(f"primT{b}", [P, 2, DIM], f32))
                 for b in range(2)]
        tmp = [tmp_ctx.enter_context(nc.sbuf_tensor(f"tmp{b}", [P, 2, DIM], f32r))
               for b in range(2)]
        # per b: lhsT buffers an_k (-A/k) and ap_k (A/k) for k=1..4
        ank = [[None] + [tmp_ctx.enter_context(
                    nc.sbuf_tensor(f"an{k}_{b}", [P, 2, DIM], f32r))
                for k in range(1, 5)] for b in range(2)]
        apk = [[None] + [tmp_ctx.enter_context(
                    nc.sbuf_tensor(f"ap{k}_{b}", [P, 2, DIM], f32r))
                for k in range(1, 5)] for b in range(2)]
        apiE = [tmp_ctx.enter_context(nc.sbuf_tensor(f"apiE{b}", [P, 2, DIM], f32r))
                for b in range(2)]
        apiT = [tmp_ctx.enter_context(nc.sbuf_tensor(f"apiT{b}", [P, 2, DIM], f32r))
                for b in range(2)]
        yE = [tmp_ctx.enter_context(nc.sbuf_tensor(f"yE{b}", [P, 2, DIM], f32r))
              for b in range(2)]
        yT = [tmp_ctx.enter_context(nc.sbuf_tensor(f"yT{b}", [P, 2, DIM], f32r))
              for b in range(2)]
        eE = [tmp_ctx.enter_context(nc.sbuf_tensor(f"eE{b}", [P, 2, DIM], f32r))
              for b in range(2)]
        eT = [tmp_ctx.enter_context(nc.sbuf_tensor(f"eT{b}", [P, 2, DIM], f32r))
              for b in range(2)]
        ci = tmp_ctx.enter_context(nc.sbuf_tensor("ci", [P, DIM], f32))
        pi = tmp_ctx.enter_context(nc.sbuf_tensor("pi", [P, 2], f32))
        identf = tmp_ctx.enter_context(nc.sbuf_tensor("identf", [P, 2, DIM], f32))
        tmp_ctx.close()
        stag_b = ctx.enter_context(nc.sbuf_tensor("stag_b", [P, NSB, 2, DIM], f16))
        stag_x = ctx.enter_context(nc.sbuf_tensor("stag_x", [P, NSX, 2, DIM], f16))
        outb = ctx.enter_context(nc.sbuf_tensor("outb", [P, NOUT, 2, DIM], f16))
        ps = [ctx.enter_context(nc.psum_tensor(f"ps{j}", [P, 2, DIM], f32))
              for j in range(8)]

        cnt = {k: 0 for k in sem}
        pe_prog, dve_prog, act_prog, gps_prog, sync_prog = [], [], [], [], []

        # ================= input DMA (sync) =================
        def s_in(s):
            s.dma_start(offs_ac[:, :], offs_ac_ext[:, :]).then_inc(sem["in_sem"], 16)
            s.dma_start(offs_gp[:, :], offs_gp_ext[:, :]).then_inc(sem["in_sem"], 16)
            for b in range(2):
                for kc in range(2):
                    s.dma_start(prim[b][:, kc, :],
                                prims_ext[b, kc * P:(kc + 1) * P, :]
                                ).then_inc(sem["in_sem"], 16)
                    s.dma_start(primT[b][:, kc, :],
                                primsT_ext[b, kc * P:(kc + 1) * P, :]
                                ).then_inc(sem["in_sem"], 16)
        sync_prog.append(s_in)
        cnt["in_sem"] = 16 * 10
        IN_ALL = cnt["in_sem"]

        # ================= identity (gpsimd iota + DVE eq) =================
        def g_iota(g):
            g.iota(ci[:, :], [[1, DIM]], channel_multiplier=0,
                   allow_small_or_imprecise_dtypes=True)
            g.iota(pi[:, 0:1], [[1, 1]], channel_multiplier=1,
                   allow_small_or_imprecise_dtypes=True)
            g.iota(pi[:, 1:2], [[1, 1]], base=P, channel_multiplier=1,
                   allow_small_or_imprecise_dtypes=True).then_inc(sem["gps_sem"], 1)
        gps_prog.append(g_iota)
        cnt["gps_sem"] += 1

        def d_ident(d, w=cnt["gps_sem"]):
            d.wait_ge(sem["gps_sem"], w)
            for kc in range(2):
                d.tensor_tensor(out=identf[:, kc, :], in0=ci[:, :],
                                in1=pi[:, kc:kc + 1].to_broadcast([P, DIM]), op=Eq)
            d.drain()
            d.tensor_copy(rn_all[:, 0, :, :], identf[:, :, :])
            d.tensor_copy(rt0[:, 0:DIM], identf[:, 0, :])
            d.tensor_copy(rt1[:, 0:DIM], identf[:, 1, :]).then_inc(sem["dve_sem"], 1)
        dve_prog.append(d_ident)
        cnt["dve_sem"] += 1
        ident_done = cnt["dve_sem"]

        # ================= expm: 4 interleaved chains =================
        # chain (b,'E'): y_k = A_k + A_k@y_{k+1}  (A_k = A/k; lhsT = -A_k)
        # chain (b,'T'): y_k = -A_k + A_k^T... generator -A: lhsT = A_k, add = -A_k
        # final e = ps + (A+I) / ps + (I-A); squarings via cross-chain lhsT.
        dve_ready = {}   # (b, tag) -> dve_sem count when ready
        pe_ready = {}    # (b, chain, round) -> pe_sem count after that matmul

        def d_pre(b):
            def fn(d, b=b, win=IN_ALL):
                d.wait_ge(sem["in_sem"], win)
                d.tensor_tensor(out=tmp[b][:, :, :], in0=prim[b][:, :, :],
                                in1=primT[b][:, :, :], op=Sub)
                d.drain()
                # what the round-0 matmuls need
                d.tensor_scalar_mul(yE[b][:, :, :], tmp[b][:, :, :], inv2s / EXPM_N)
                d.tensor_scalar_mul(yT[b][:, :, :], tmp[b][:, :, :], -inv2s / EXPM_N)
                d.tensor_scalar_mul(ank[b][4][:, :, :], tmp[b][:, :, :], -inv2s / 4)
                d.tensor_scalar_mul(apk[b][4][:, :, :], tmp[b][:, :, :],
                                    inv2s / 4).then_inc(sem["dve_sem"], 1)
            return fn

        for b in range(2):
            dve_prog.append(d_pre(b))
            cnt["dve_sem"] += 1
            dve_ready[(b, "r0")] = cnt["dve_sem"]

        def d_pre2(b):
            def fn(d, b=b):
                for k in (3, 2, 1):
                    d.tensor_scalar_mul(ank[b][k][:, :, :], tmp[b][:, :, :],
                                        -inv2s / k)
                    d.tensor_scalar_mul(apk[b][k][:, :, :], tmp[b][:, :, :],
                                        inv2s / k)
                d.drain()
                d.tensor_tensor(out=apiE[b][:, :, :], in0=apk[b][1][:, :, :],
                                in1=identf[:, :, :], op=Add)
                d.tensor_tensor(out=apiT[b][:, :, :], in0=ank[b][1][:, :, :],
                                in1=identf[:, :, :],
                                op=Add).then_inc(sem["dve_sem"], 1)
            return fn

        for b in range(2):
            dve_prog.append(d_pre2(b))
            cnt["dve_sem"] += 1
            dve_ready[(b, "pre2")] = cnt["dve_sem"]

        chains = [(0, "E"), (0, "T"), (1, "E"), (1, "T")]
        chain_ps = {c: i for i, c in enumerate(chains)}

        # PE Horner rounds r=0..3 (k = 4-r), then sq1 (all), sq2 (E only)
        for r in range(4):
            k = 4 - r
            for c in chains:
                b, ch = c
                if r == 0:
                    wd = dve_ready[(b, "r0")]
                elif r == 3:
                    # k=1 uses apk[1]/ank[1] (pre2) and y from round 2
                    wd = max(dve_ready[(b, "pre2")], dve_ready[(b, c, r - 1)])
                else:
                    wd = max(dve_ready[(b, "pre2")] if k < 4 else 0,
                             dve_ready[(b, c, r - 1)])

                def p_h(t, b=b, ch=ch, k=k, wd=wd, bank=chain_ps[(b, ch)]):
                    t.wait_ge(sem["dve_sem"], wd)
                    lhs = ank[b][k] if ch == "E" else apk[b][k]
                    y = yE[b] if ch == "E" else yT[b]
                    last = None
                    for mc in range(2):
                        for kc in range(2):
                            last = t.matmul(ps[bank][:, mc, :],
                                            lhs[:, kc, mc * P:(mc + 1) * P],
                                            y[:, kc, :],
                                            start=(kc == 0), stop=(kc == 1))
                    last.then_inc(sem["pe_sem"], 1)
                pe_prog.append(p_h)
                cnt["pe_sem"] += 1
                pe_ready[(b, c, r)] = cnt["pe_sem"]

                # DVE add for this chain/round
                if r < 3:
                    def d_h(d, b=b, ch=ch, k=k, wp=cnt["pe_sem"],
                            bank=chain_ps[(b, ch)]):
                        d.wait_ge(sem["pe_sem"], wp)
                        y = yE[b] if ch == "E" else yT[b]
                        # y_k = A_k @ y_{k+1} + A_k  (chain T: ... - A_k)
                        if ch == "E":
                            d.tensor_tensor(out=y[:, :, :], in0=ps[bank][:, :, :],
                                            in1=apk[b][k][:, :, :],
                                            op=Add).then_inc(sem["dve_sem"], 1)
                        else:
                            d.tensor_tensor(out=y[:, :, :], in0=ps[bank][:, :, :],
                                            in1=apk[b][k][:, :, :],
                                            op=Sub).then_inc(sem["dve_sem"], 1)
                else:
                    # k=1: e = ps + (I +/- A)
                    def d_h(d, b=b, ch=ch, wp=cnt["pe_sem"],
                            bank=chain_ps[(b, ch)]):
                        d.wait_ge(sem["pe_sem"], wp)
                        e = eE[b] if ch == "E" else eT[b]
                        api = apiE[b] if ch == "E" else apiT[b]
                        d.tensor_tensor(out=e[:, :, :], in0=ps[bank][:, :, :],
                                        in1=api[:, :, :],
                                        op=Add).then_inc(sem["dve_sem"], 1)
                dve_prog.append(d_h)
                cnt["dve_sem"] += 1
                dve_ready[(b, c, r)] = cnt["dve_sem"]

        # NOTE on the "add" for chain T rounds: y_new = -A_k + ps. tensor_tensor
        # Sub computes in0 - in1 = ps - A_k = ps + (-A_k). Correct.

        # sq1: E2 = E@E (lhsT = eT), T2 = ET@ET (lhsT = eE); out -> y buffers
        for c in chains:
            b, ch = c
            wd = max(dve_ready[(b, (b, "E"), 3)], dve_ready[(b, (b, "T"), 3)])

            def p_sq1(t, b=b, ch=ch, wd=wd, bank=chain_ps[(b, ch)]):
                t.wait_ge(sem["dve_sem"], wd)
                lhs = eT[b] if ch == "E" else eE[b]
                rhs = eE[b] if ch == "E" else eT[b]
                last = None
                for mc in range(2):
                    for kc in range(2):
                        last = t.matmul(ps[bank][:, mc, :],
                                        lhs[:, kc, mc * P:(mc + 1) * P],
                                        rhs[:, kc, :],
                                        start=(kc == 0), stop=(kc == 1))
                last.then_inc(sem["pe_sem"], 1)
            pe_prog.append(p_sq1)
            cnt["pe_sem"] += 1

            def d_sq1(d, b=b, ch=ch, wp=cnt["pe_sem"], bank=chain_ps[(b, ch)]):
                d.wait_ge(sem["pe_sem"], wp)
                dst = yE[b] if ch == "E" else yT[b]   # reuse y as E-level-1
                d.tensor_copy(dst[:, :, :], ps[bank][:, :, :]).then_inc(
                    sem["dve_sem"], 1)
            dve_prog.append(d_sq1)
            cnt["dve_sem"] += 1
            dve_ready[(b, c, "sq1")] = cnt["dve_sem"]

        # sq2 (E chains only): P_b = E1@E1 (lhsT = yT[b] = E1^T), then pbf cast
        pbf_done = {}
        for b in range(2):
            wd = max(dve_ready[(b, (b, "E"), "sq1")], dve_ready[(b, (b, "T"), "sq1")])

            def p_sq2(t, b=b, wd=wd, bank=chain_ps[(b, "E")]):
                t.wait_ge(sem["dve_sem"], wd)
                last = None
                for mc in range(2):
                    for kc in range(2):
                        last = t.matmul(ps[bank][:, mc, :],
                                        yT[b][:, kc, mc * P:(mc + 1) * P],
                                        yE[b][:, kc, :],
                                        start=(kc == 0), stop=(kc == 1))
                last.then_inc(sem["pe_sem"], 1)
            pe_prog.append(p_sq2)
            cnt["pe_sem"] += 1

            def d_pbf(d, b=b, wp=cnt["pe_sem"], bank=chain_ps[(b, "E")]):
                d.wait_ge(sem["pe_sem"], wp)
                d.tensor_copy(pbf[:, b, :, :], ps[bank][:, :, :]).then_inc(
                    sem["dve_sem"], 1)
            dve_prog.append(d_pbf)
            cnt["dve_sem"] += 1
            pbf_done[b] = cnt["dve_sem"]

        # ================= table build =================
        build_items = [("n", q) for q in range(2, 64)] + \
                      [("t", q) for q in range(64, 128)]
        bank_owner = {}
        entry_done = {("n", 1): ("dve_sem", ident_done)}

        for j, (kind, q) in enumerate(build_items):
            bank = j % 8
            b = q & 1
            par = q >> 1

            waits = [("dve_sem", pbf_done[b]), entry_done[("n", par)]]
            if bank in bank_owner:
                waits.append(bank_owner[bank])
            elif bank in (2, 3):
                # banks 2/3 held by b=1 expm chains until the pbf[1] cast
                waits.append(("dve_sem", pbf_done[1]))

            def p_build(t, kind=kind, b=b, par=par, bank=bank, waits=tuple(waits)):
                for s_, c_ in waits:
                    t.wait_ge(sem[s_], c_)
                last = None
                for mc in range(2):
                    for kc in range(2):
                        if kind == "n":
                            lhsT = pbf[:, b, kc, mc * P:(mc + 1) * P]
                            rhs = rn_all[:, par - 1, kc, :]
                        else:
                            lhsT = rn_all[:, par - 1, kc, mc * P:(mc + 1) * P]
                            rhs = pbf[:, b, kc, :]
                        last = t.matmul(ps[bank][:, mc, :], lhsT, rhs,
                                        start=(kc == 0), stop=(kc == 1))
                last.then_inc(sem["pe_sem"], 1)
            pe_prog.append(p_build)
            cnt["pe_sem"] += 1

            ceng = "dve_sem" if j % 2 == 0 else "act_sem"
            prog = dve_prog if j % 2 == 0 else act_prog

            def x_copy(e, kind=kind, q=q, bank=bank, w=cnt["pe_sem"], ceng=ceng):
                e.wait_ge(sem["pe_sem"], w)
                if kind == "n":
                    if ceng == "dve_sem":
                        e.tensor_copy(rn_all[:, q - 1, :, :],
                                      ps[bank][:, :, :]).then_inc(sem[ceng], 1)
                    else:
                        e.mul(rn_all[:, q - 1, :, :],
                              ps[bank][:, :, :], 1.0).then_inc(sem[ceng], 1)
                else:
                    slot = q - 63
                    if ceng == "dve_sem":
                        e.tensor_copy(rt0[:, slot * DIM:(slot + 1) * DIM],
                                      ps[bank][:, 0, :])
                        e.tensor_copy(rt1[:, slot * DIM:(slot + 1) * DIM],
                                      ps[bank][:, 1, :]).then_inc(sem[ceng], 1)
                    else:
                        e.mul(rt0[:, slot * DIM:(slot + 1) * DIM],
                              ps[bank][:, 0, :], 1.0)
                        e.mul(rt1[:, slot * DIM:(slot + 1) * DIM],
                              ps[bank][:, 1, :], 1.0).then_inc(sem[ceng], 1)
            prog.append(x_copy)
            cnt[ceng] += 1
            entry_done[(kind, q)] = (ceng, cnt[ceng])
            bank_owner[bank] = (ceng, cnt[ceng])

        build_dve = cnt["dve_sem"]
        build_act = cnt["act_sem"]

        # ================= position phase =================
        def g_pos(g, bd=build_dve, ba=build_act):
            g.wait_ge(sem["in_sem"], IN_ALL)
            g.wait_ge(sem["dve_sem"], bd)
            g.wait_ge(sem["act_sem"], ba)
            regs = [g.alloc_register(f"gb{x}") for x in range(GB)]
            for kb in range(NGB):
                g.reg_load(regs, offs_gp[kb:kb + 1, 0:GB])
                for j in range(GB):
                    i = GB * kb + j
                    if i >= NSB:
                        g.wait_ge(sem["mm1_sem"], i - NSB + 1)
                    src = bass.AP(rn_all, regs[j], [[NSTRIDE, P], [DIM, 2], [1, DIM]])
                    g.dma_start(stag_b[:, i % NSB, :, :], src).then_inc(
                        sem["stg_sem"], 16)
            for r in regs:
                g.free_register(r)
        gps_prog.append(g_pos)

        def p_pos(t, bd=build_dve, ba=build_act):
            t.wait_ge(sem["in_sem"], IN_ALL)
            t.wait_ge(sem["dve_sem"], bd)
            t.wait_ge(sem["act_sem"], ba)
            regs = [t.alloc_register(f"pr{x}") for x in range(48)]
            for k in range(NB + 1):
                # row k: [A offsets batch k (16) | C offset pairs batch k-1 (32)]
                # (TensorLoad caps at 32 registers per instruction)
                t.reg_load(regs[0:16], offs_ac[k:k + 1, 0:16])
                t.reg_load(regs[16:48], offs_ac[k:k + 1, 16:48])
                for g in range(4):
                    if k < NB:
                        i0 = BATCH * k + 4 * g
                        if i0 >= 4:
                            t.wait_ge(sem["dvex_sem"], i0)
                        t.wait_ge(sem["stg_sem"], 16 * (i0 + 4))
                        for u in range(4):
                            i = i0 + u
                            ra = regs[4 * g + u]
                            last = None
                            for mc in range(2):
                                for kc in range(2):
                                    rt = rt0 if kc == 0 else rt1
                                    rhs = bass.AP(rt, ra, [[TSTRIDE, P], [1, DIM]])
                                    last = t.matmul(
                                        ps[i % 4][:, mc, :],
                                        stag_b[:, i % NSB, kc, mc * P:(mc + 1) * P],
                                        rhs, start=(kc == 0), stop=(kc == 1))
                            last.then_inc(sem["mm1_sem"], 1)
                    if k >= 1:
                        j0 = BATCH * (k - 1) + 4 * g
                        t.wait_ge(sem["dvex_sem"], j0 + 4)
                        if j0 >= 4:
                            t.wait_ge(sem["actp_sem"], j0)
                        for u in range(4):
                            jj = j0 + u
                            jb = 4 * g + u
                            last = None
                            for mc in range(2):
                                for kc in range(2):
                                    rc = regs[16 + 2 * jb + kc]
                                    rhs = bass.AP(rn_all, rc, [[NSTRIDE, P], [1, DIM]])
                                    last = t.matmul(
                                        ps[4 + jj % 4][:, mc, :],
                                        stag_x[:, jj % NSX, kc, mc * P:(mc + 1) * P],
                                        rhs, start=(kc == 0), stop=(kc == 1))
                            last.then_inc(sem["mm2_sem"], 1)
            for r in regs:
                t.free_register(r)
        pe_prog.append(p_pos)

        def d_pos(d):
            for i in range(npos):
                d.wait_ge(sem["mm1_sem"], i + 1)
                if i >= NSX:
                    d.wait_ge(sem["mm2_sem"], i - NSX + 1)
                d.tensor_copy(stag_x[:, i % NSX, :, :],
                              ps[i % 4][:, :, :]).then_inc(sem["dvex_sem"], 1)
        dve_prog.append(d_pos)

        def a_pos(a):
            for i in range(npos):
                m = i // 4
                a.wait_ge(sem["mm2_sem"], i + 1)
                if i % 4 == 0 and m >= 2:
                    a.wait_ge(sem[f"dma_s{m % 2}"], 16 * (m // 2))
                a.mul(outb[:, i % NOUT, :, :],
                      ps[4 + i % 4][:, :, :], 1.0).then_inc(sem["actp_sem"], 1)
        act_prog.append(a_pos)

        if dump:
            def s_dump(s, bd=build_dve, ba=build_act):
                s.wait_ge(sem["dve_sem"], bd)
                s.wait_ge(sem["act_sem"], ba)
                s.dma_start(bass.AP(dbg_rn, 0, [[NAT_E * 2 * DIM, P],
                                                [1, NAT_E * 2 * DIM]]),
                            bass.AP(rn_all, 0, [[NSTRIDE, P], [1, NSTRIDE]])
                            ).then_inc(sem["in_sem"], 16)
                s.dma_start(dbg_rt0[:, :], rt0[:, :]).then_inc(sem["in_sem"], 16)
                s.dma_start(dbg_rt1[:, :], rt1[:, :]).then_inc(sem["in_sem"], 16)
                s.dma_start(dbg_pbf[:, :],
                            bass.AP(pbf, 0, [[2 * 2 * DIM, P], [1, 2 * 2 * DIM]])
                            ).then_inc(sem["in_sem"], 16)
                s.wait_ge(sem["in_sem"], IN_ALL + 64)
            sync_prog.append(s_dump)

        def s_pos(s):
            for m in range(NGRP):
                s.wait_ge(sem["actp_sem"], 4 * (m + 1))
                dst = bass.AP(out_ext, m * 4 * 2 * P * DIM,
                              [[DIM, P], [2 * P * DIM, 4], [P * DIM, 2], [1, DIM]])
                src = outb[:, (m % 2) * 4:(m % 2) * 4 + 4, :, :]
                s.dma_start(dst, src).then_inc(sem[f"dma_s{m % 2}"], 16)
            s.wait_ge(sem["dma_s0"], 16 * ((NGRP + 1) // 2))
            s.wait_ge(sem["dma_s1"], 16 * (NGRP // 2))
        sync_prog.append(s_pos)

        # ================= emit =================
        with nc.Block() as block:
            @block.tensor
            def _(tensor):
                for fn in pe_prog:
                    fn(tensor)

            @block.vector
            def _(vector):
                for fn in dve_prog:
                    fn(vector)

            @block.scalar
            def _(scalar):
                for fn in act_prog:
                    fn(scalar)

            @block.gpsimd
            def _(gpsimd):
                for fn in gps_prog:
                    fn(gpsimd)

            @block.sync
            def _(sync):
                for fn in sync_prog:
                    fn(sync)

    return nc


def _host_offsets(u):
    """u: (n,) int64 positions -> per-position table offsets (oA, oB, oC0, oC1).

    oA: element offset into rt0/rt1 (slot*DIM); oB: offset into rn_all
    (slot*ENT) for the stag_b gather; oC0/oC1: rn_all offsets for the two
    K-passes of mm2."""
    u = u.astype(np.int64)
    blen = np.zeros_like(u)
    t = u.copy()
    while np.any(t > 0):
        blen = np.where(t > 0, blen + 1, blen)
        t >>= 1
    k = blen - 1  # path length
    tA = np.minimum(k, 6)
    idxA = (1 << tA) + (u & ((1 << tA) - 1))
    tB = np.clip(k - 6, 0, 5)
    idxB = (1 << tB) + ((u >> 6) & ((1 << tB) - 1))
    tC = np.clip(k - 11, 0, 5)
    idxC = (1 << tC) + ((u >> 11) & ((1 << tC) - 1))
    short = u < 64
    idxA = np.where(short, 1, idxA)
    idxB = np.where(short, u, idxB)
    assert idxA.max() < 128 and idxB.max() < 64 and idxC.max() < 64
    assert np.all((idxA == 1) | (idxA >= 64))
    slotA = np.where(idxA == 1, 0, idxA - 63)
    oA = slotA * DIM
    oB = (idxB - 1) * ENT
    oC0 = (idxC - 1) * ENT
    return (oA.astype(np.int32), oB.astype(np.int32),
            oC0.astype(np.int32), (oC0 + DIM).astype(np.int32))


def kernel(primitives, identity, unique):
    global LAST_RESULTS
    from concourse.bass_utils import run_bass_kernel_spmd

    prims = np.ascontiguousarray(np.asarray(primitives, dtype=np.float32))
    primsT = np.ascontiguousarray(prims.transpose(0, 2, 1))
    u = np.asarray(unique).astype(np.int64).ravel()
    n = u.shape[0]
    assert n % NCORES == 0
    npos = n // NCORES
    NB = npos // BATCH
    NGB = npos // GB

    oA, oB, oC0, oC1 = _host_offsets(u)

    if npos not in _NC_CACHE:
        nc = _build_nc(npos)
        nc.compile()
        _NC_CACHE[npos] = nc
    nc = _NC_CACHE[npos]

    in_maps = []
    for c in range(NCORES):
        sl = slice(c * npos, (c + 1) * npos)
        a = oA[sl].reshape(NB, BATCH)
        c0 = oC0[sl].reshape(NB, BATCH)
        c1 = oC1[sl].reshape(NB, BATCH)
        ac = np.zeros((P, 48), np.int32)
        for k in range(NB + 1):
            if k < NB:
                ac[k, 0:BATCH] = a[k]
            if k >= 1:
                ac[k, BATCH::2] = c0[k - 1]
                ac[k, BATCH + 1::2] = c1[k - 1]
        gp = np.zeros((P, GB), np.int32)
        gp[:NGB, :] = oB[sl].reshape(NGB, GB)
        in_maps.append({"prims": prims, "primsT": primsT,
                        "offs_ac": ac, "offs_gp": gp})

    import os
    trace_dir = os.environ.get("KERNEL_TRACE_DIR")
    res = run_bass_kernel_spmd(nc, in_maps, core_ids=list(range(NCORES)),
                               tmpdir=trace_dir)
    LAST_RESULTS = res

    parts = []
    for c in range(NCORES):
        o = np.asarray(res.results[c]["out"])  # (npos, 2, 128, 256) f16
        parts.append(o.reshape(npos, DIM, DIM).astype(np.float32))
    out = np.concatenate(parts, axis=0)

    ident = np.asarray(identity, dtype=np.float32)[0]
    if not np.allclose(ident, np.eye(DIM, dtype=np.float32)):
        out = np.einsum("ij,njk->nik", ident, out).astype(np.float32)
    return out


# revision 12
# speedup vs baseline: 1.3426x; 1.3426x over previous
"""Trainium2 Bass kernel for nn_BinaryPathEncoder (v3).

Math: for each position p, R(p) = M_{b0} @ M_{b1} @ ... (LSB-first binary
path, leading 1 stripped), M_b = expm(B_b - B_b^T)^T.  Split the <=16-step
path 6+5+5 so R(p) = R_A @ R_B @ R_C with f16 table lookups:
  mm1: X1T = (R_A R_B)^T = matmul(lhsT=stag_b, rhs=stag_a)     [all static]
  mm2: O   = R_A R_B R_C  = matmul(lhsT=stag_x, rhs=Rn[idxC])  [snap-value AP]
expm on device: f32r Taylor-Horner (S=2/N=5) with pre-scaled A/k buffers,
four interleaved chains, transpose-free squarings.

Position-phase design (per core, npos=256):
- mm1 operands are DMA-staged: A entries gathered per position (gpsimd,
  register-offset src APs, batched reg loads), B entries staged by the
  sync queue with host-side dedup: positions are sorted by idxB and
  arranged as 96 same-B pairs + 64 singles (always feasible), so only
  160 B stage DMAs.  The PE's mm1 instructions are fully static ->
  109ns/matmul streaming.
- mm2's moving operand reads the natural table via snap()-materialized
  ScalarValue AP offsets (walrus encodes those inline in $R[8]/$R[9] with
  no per-matmul prefix, unlike raw RegisterHandle APs).  Two 16-bit
  offsets per word, +DIM values derived with snap adds.
- Output: [npos, P, 2, DIM] layout (1KB contiguous runs per partition),
  one dma_start per 4 positions.
"""

import contextlib
import numpy as np

DIM = 256
NCORES = 8
P = 128

NAT_E = 63                     # natural table entries (q in [1,64))
TRA_E = 65                     # transposed: slot0=identity, 1..64 = q in [64,128)
ENT = 512                      # entry elements per partition (2 kc x 256)
NSTRIDE = NAT_E * ENT          # per-partition stride of rn_all
TSTRIDE = TRA_E * ENT          # per-partition stride of rt_all

BATCH = 16                     # PE batch (one C-offset reg_load per batch)
GB = 8                         # gather reg_load batch
NSA = 16                       # stag_a ring slots
NSB = 16                       # stag_b ring slots
NSX = 16                       # stag_x ring slots
NOUT = 8                       # outb slots (2 dma groups of 4)
NPAIR_B = 96                   # same-B pairs (2 positions / stage)
EXPM_S = 2
EXPM_N = 5

_NC_CACHE = {}
LAST_RESULTS = None


def _bslot(p):
    """static stag_b slot for (sorted) position p"""
    s = p // 2 if p < 2 * NPAIR_B else NPAIR_B + (p - 2 * NPAIR_B)
    return s


def _blast(s, npos):
    """static last position covered by B stage s"""
    return 2 * s + 1 if s < NPAIR_B else 2 * NPAIR_B + (s - NPAIR_B)


def _build_nc(npos, debug=False, dump=False):
    from concourse import bass, bacc, mybir

    f32 = mybir.dt.float32
    f32r = mybir.dt.float32r
    f16 = mybir.dt.float16
    i32 = mybir.dt.int32
    Sub = mybir.AluOpType.subtract
    Add = mybir.AluOpType.add
    Eq = mybir.AluOpType.is_equal
    And = mybir.AluOpType.bitwise_and
    Shr = mybir.AluOpType.logical_shift_right

    assert npos % BATCH == 0
    NB = npos // BATCH
    NGB = npos // GB
    NGRP = npos // 4
    NSTAGE_B = NPAIR_B + (npos - 2 * NPAIR_B)   # 160 for npos=256
    assert npos - 2 * NPAIR_B >= 0

    nc = bacc.Bacc("TRN2", target_bir_lowering=False, debug=debug)

    prims_ext = nc.dram_tensor("prims", [2, DIM, DIM], f32, kind="ExternalInput")
    primsT_ext = nc.dram_tensor("primsT", [2, DIM, DIM], f32, kind="ExternalInput")
    offs_a_ext = nc.dram_tensor("offs_a", [P, GB], i32, kind="ExternalInput")
    offs_b_ext = nc.dram_tensor("offs_b", [P, GB], i32, kind="ExternalInput")
    offs_c_ext = nc.dram_tensor("offs_c", [P, GB], i32, kind="ExternalInput")
    out_ext = nc.dram_tensor("out", [npos, P, 2, DIM], f16, kind="ExternalOutput")
    if dump:
        dbg_rn = nc.dram_tensor("dbg_rn", [P, NSTRIDE], f16, kind="ExternalOutput")
        dbg_rt = nc.dram_tensor("dbg_rt", [P, TSTRIDE], f16, kind="ExternalOutput")
        dbg_pbf = nc.dram_tensor("dbg_pbf", [P, 2 * 2 * DIM], f16,
                                 kind="ExternalOutput")

    inv2s = 1.0 / (2.0 ** EXPM_S)

    with contextlib.ExitStack() as ctx:
        sem = {}
        for name in ["in_sem", "gps_sem", "pe_sem", "dve_sem", "act_sem",
                     "mm1_sem", "mm2_sem", "dvex_sem", "actp_sem",
                     "stga_sem", "stgb_sem", "dma_s0", "dma_s1"]:
            sem[name] = ctx.enter_context(nc.semaphore(name))

        # ---- persistent SBUF ----
        rt_all = ctx.enter_context(nc.sbuf_tensor("rt_all", [P, TRA_E, 2, DIM], f16))
        rn_all = ctx.enter_context(nc.sbuf_tensor("rn_all", [P, NAT_E, 2, DIM], f16))
        pbf = ctx.enter_context(nc.sbuf_tensor("pbf", [P, 2, 2, DIM], f16))
        offs_a = ctx.enter_context(nc.sbuf_tensor("offs_a_sb", [P, GB], i32))
        offs_b = ctx.enter_context(nc.sbuf_tensor("offs_b_sb", [P, GB], i32))
        offs_c = ctx.enter_context(nc.sbuf_tensor("offs_c_sb", [P, GB], i32))

        # ---- expm-phase buffers; space reused by position-phase staging ----
        tmp_ctx = contextlib.ExitStack()
        prim = [tmp_ctx.enter_context(nc.sbuf_tensor(f"prim{b}", [P, 2, DIM], f32))
                for b in range(2)]
        primT = [tmp_ctx.enter_context(nc.sbuf_tensor(f"primT{b}", [P, 2, DIM], f32))
                 for b in range(2)]
        tmp = [tmp_ctx.enter_context(nc.sbuf_tensor(f"tmp{b}", [P, 2, DIM], f32r))
               for b in range(2)]
        ank = [[None] + [tmp_ctx.enter_context(
                    nc.sbuf_tensor(f"an{k}_{b}", [P, 2, DIM], f32r))
                for k in range(1, 5)] for b in range(2)]
        apk = [[None] + [tmp_ctx.enter_context(
                    nc.sbuf_tensor(f"ap{k}_{b}", [P, 2, DIM], f32r))
                for k in range(1, 5)] for b in range(2)]
        apiE = [tmp_ctx.enter_context(nc.sbuf_tensor(f"apiE{b}", [P, 2, DIM], f32r))
                for b in range(2)]
        apiT = [tmp_ctx.enter_context(nc.sbuf_tensor(f"apiT{b}", [P, 2, DIM], f32r))
                for b in range(2)]
        yE = [tmp_ctx.enter_context(nc.sbuf_tensor(f"yE{b}", [P, 2, DIM], f32r))
              for b in range(2)]
        yT = [tmp_ctx.enter_context(nc.sbuf_tensor(f"yT{b}", [P, 2, DIM], f32r))
              for b in range(2)]
        eE = [tmp_ctx.enter_context(nc.sbuf_tensor(f"eE{b}", [P, 2, DIM], f32r))
              for b in range(2)]
        eT = [tmp_ctx.enter_context(nc.sbuf_tensor(f"eT{b}", [P, 2, DIM], f32r))
              for b in range(2)]
        ci = tmp_ctx.enter_context(nc.sbuf_tensor("ci", [P, DIM], f32))
        pi = tmp_ctx.enter_context(nc.sbuf_tensor("pi", [P, 2], f32))
        identf = tmp_ctx.enter_context(nc.sbuf_tensor("identf", [P, 2, DIM], f32))
        tmp_ctx.close()
        stag_a = ctx.enter_context(nc.sbuf_tensor("stag_a", [P, NSA, 2, DIM], f16))
        stag_b = ctx.enter_context(nc.sbuf_tensor("stag_b", [P, NSB, 2, DIM], f16))
        stag_x = ctx.enter_context(nc.sbuf_tensor("stag_x", [P, NSX, 2, DIM], f16))
        outb = ctx.enter_context(nc.sbuf_tensor("outb", [P, NOUT, 2, DIM], f16))
        ps = [ctx.enter_context(nc.psum_tensor(f"ps{j}", [P, 2, DIM], f32))
              for j in range(8)]

        cnt = {k: 0 for k in sem}
        pe_prog, dve_prog, act_prog, gps_prog, sync_prog = [], [], [], [], []

        # ================= input DMA (sync) =================
        def s_in(s):
            s.dma_start(offs_a[:, :], offs_a_ext[:, :]).then_inc(sem["in_sem"], 16)
            s.dma_start(offs_b[:, :], offs_b_ext[:, :]).then_inc(sem["in_sem"], 16)
            s.dma_start(offs_c[:, :], offs_c_ext[:, :]).then_inc(sem["in_sem"], 16)
            for b in range(2):
                for kc in range(2):
                    s.dma_start(prim[b][:, kc, :],
                                prims_ext[b, kc * P:(kc + 1) * P, :]
                                ).then_inc(sem["in_sem"], 16)
                    s.dma_start(primT[b][:, kc, :],
                                primsT_ext[b, kc * P:(kc + 1) * P, :]
                                ).then_inc(sem["in_sem"], 16)
        sync_prog.append(s_in)
        cnt["in_sem"] = 16 * 11
        IN_ALL = cnt["in_sem"]

        # ================= identity (gpsimd iota + DVE eq) =================
        def g_iota(g):
            g.iota(ci[:, :], [[1, DIM]], channel_multiplier=0,
                   allow_small_or_imprecise_dtypes=True)
            g.iota(pi[:, 0:1], [[1, 1]], channel_multiplier=1,
                   allow_small_or_imprecise_dtypes=True)
            g.iota(pi[:, 1:2], [[1, 1]], base=P, channel_multiplier=1,
                   allow_small_or_imprecise_dtypes=True).then_inc(sem["gps_sem"], 1)
        gps_prog.append(g_iota)
        cnt["gps_sem"] += 1

        def d_ident(d, w=cnt["gps_sem"]):
            d.wait_ge(sem["gps_sem"], w)
            for kc in range(2):
                d.tensor_tensor(out=identf[:, kc, :], in0=ci[:, :],
                                in1=pi[:, kc:kc + 1].to_broadcast([P, DIM]), op=Eq)
            d.drain()
            d.tensor_copy(rn_all[:, 0, :, :], identf[:, :, :])
            d.tensor_copy(rt_all[:, 0, :, :], identf[:, :, :]).then_inc(
                sem["dve_sem"], 1)
        dve_prog.append(d_ident)
        cnt["dve_sem"] += 1
        ident_done = cnt["dve_sem"]

        # ================= expm: 4 interleaved chains =================
        dve_ready = {}

        def d_pre(b):
            def fn(d, b=b, win=IN_ALL):
                d.wait_ge(sem["in_sem"], win)
                d.tensor_tensor(out=tmp[b][:, :, :], in0=prim[b][:, :, :],
                                in1=primT[b][:, :, :], op=Sub)
                d.drain()
                d.tensor_scalar_mul(yE[b][:, :, :], tmp[b][:, :, :], inv2s / EXPM_N)
                d.tensor_scalar_mul(yT[b][:, :, :], tmp[b][:, :, :], -inv2s / EXPM_N)
                d.tensor_scalar_mul(ank[b][4][:, :, :], tmp[b][:, :, :], -inv2s / 4)
                d.tensor_scalar_mul(apk[b][4][:, :, :], tmp[b][:, :, :],
                                    inv2s / 4).then_inc(sem["dve_sem"], 1)
            return fn

        for b in range(2):
            dve_prog.append(d_pre(b))
            cnt["dve_sem"] += 1
            dve_ready[(b, "r0")] = cnt["dve_sem"]

        def d_pre2(b):
            def fn(d, b=b):
                for k in (3, 2, 1):
                    d.tensor_scalar_mul(ank[b][k][:, :, :], tmp[b][:, :, :],
                                        -inv2s / k)
                    d.tensor_scalar_mul(apk[b][k][:, :, :], tmp[b][:, :, :],
                                        inv2s / k)
                d.drain()
                d.tensor_tensor(out=apiE[b][:, :, :], in0=apk[b][1][:, :, :],
                                in1=identf[:, :, :], op=Add)
                d.tensor_tensor(out=apiT[b][:, :, :], in0=ank[b][1][:, :, :],
                                in1=identf[:, :, :],
                                op=Add).then_inc(sem["dve_sem"], 1)
            return fn

        for b in range(2):
            dve_prog.append(d_pre2(b))
            cnt["dve_sem"] += 1
            dve_ready[(b, "pre2")] = cnt["dve_sem"]

        chains = [(0, "E"), (0, "T"), (1, "E"), (1, "T")]
        chain_ps = {c: i for i, c in enumerate(chains)}

        for r in range(4):
            k = 4 - r
            for c in chains:
                b, ch = c
                if r == 0:
                    wd = dve_ready[(b, "r0")]
                elif r == 3:
                    wd = max(dve_ready[(b, "pre2")], dve_ready[(b, c, r - 1)])
                else:
                    wd = max(dve_ready[(b, "pre2")], dve_ready[(b, c, r - 1)])

                def p_h(t, b=b, ch=ch, k=k, wd=wd, bank=chain_ps[(b, ch)]):
                    t.wait_ge(sem["dve_sem"], wd)
                    lhs = ank[b][k] if ch == "E" else apk[b][k]
                    y = yE[b] if ch == "E" else yT[b]
                    last = None
                    for mc in range(2):
                        for kc in range(2):
                            last = t.matmul(ps[bank][:, mc, :],
                                            lhs[:, kc, mc * P:(mc + 1) * P],
                                            y[:, kc, :],
                                            start=(kc == 0), stop=(kc == 1))
                    last.then_inc(sem["pe_sem"], 1)
                pe_prog.append(p_h)
                cnt["pe_sem"] += 1

                if r < 3:
                    def d_h(d, b=b, ch=ch, k=k, wp=cnt["pe_sem"],
                            bank=chain_ps[(b, ch)]):
                        d.wait_ge(sem["pe_sem"], wp)
                        y = yE[b] if ch == "E" else yT[b]
                        d.tensor_tensor(out=y[:, :, :], in0=ps[bank][:, :, :],
                                        in1=apk[b][k][:, :, :],
                                        op=(Add if ch == "E" else Sub)
                                        ).then_inc(sem["dve_sem"], 1)
                else:
                    def d_h(d, b=b, ch=ch, wp=cnt["pe_sem"],
                            bank=chain_ps[(b, ch)]):
                        d.wait_ge(sem["pe_sem"], wp)
                        e = eE[b] if ch == "E" else eT[b]
                        api = apiE[b] if ch == "E" else apiT[b]
                        d.tensor_tensor(out=e[:, :, :], in0=ps[bank][:, :, :],
                                        in1=api[:, :, :],
                                        op=Add).then_inc(sem["dve_sem"], 1)
                dve_prog.append(d_h)
                cnt["dve_sem"] += 1
                dve_ready[(b, c, r)] = cnt["dve_sem"]

        # sq1 (both chains), out -> y buffers (free after Horner)
        for c in chains:
            b, ch = c
            wd = max(dve_ready[(b, (b, "E"), 3)], dve_ready[(b, (b, "T"), 3)])

            def p_sq1(t, b=b, ch=ch, wd=wd, bank=chain_ps[(b, ch)]):
                t.wait_ge(sem["dve_sem"], wd)
                lhs = eT[b] if ch == "E" else eE[b]
                rhs = eE[b] if ch == "E" else eT[b]
                last = None
                for mc in range(2):
                    for kc in range(2):
                        last = t.matmul(ps[bank][:, mc, :],
                                        lhs[:, kc, mc * P:(mc + 1) * P],
                                        rhs[:, kc, :],
                                        start=(kc == 0), stop=(kc == 1))
                last.then_inc(sem["pe_sem"], 1)
            pe_prog.append(p_sq1)
            cnt["pe_sem"] += 1

            def d_sq1(d, b=b, ch=ch, wp=cnt["pe_sem"], bank=chain_ps[(b, ch)]):
                d.wait_ge(sem["pe_sem"], wp)
                dst = yE[b] if ch == "E" else yT[b]
                d.tensor_copy(dst[:, :, :], ps[bank][:, :, :]).then_inc(
                    sem["dve_sem"], 1)
            dve_prog.append(d_sq1)
            cnt["dve_sem"] += 1
            dve_ready[(b, c, "sq1")] = cnt["dve_sem"]

        # sq2 (E chains only): P_b -> pbf f16 straight from PSUM
        pbf_done = {}
        for b in range(2):
            wd = max(dve_ready[(b, (b, "E"), "sq1")],
                     dve_ready[(b, (b, "T"), "sq1")])

            def p_sq2(t, b=b, wd=wd, bank=chain_ps[(b, "E")]):
                t.wait_ge(sem["dve_sem"], wd)
                last = None
                for mc in range(2):
                    for kc in range(2):
                        last = t.matmul(ps[bank][:, mc, :],
                                        yT[b][:, kc, mc * P:(mc + 1) * P],
                                        yE[b][:, kc, :],
                                        start=(kc == 0), stop=(kc == 1))
                last.then_inc(sem["pe_sem"], 1)
            pe_prog.append(p_sq2)
            cnt["pe_sem"] += 1

            def d_pbf(d, b=b, wp=cnt["pe_sem"], bank=chain_ps[(b, "E")]):
                d.wait_ge(sem["pe_sem"], wp)
                d.tensor_copy(pbf[:, b, :, :], ps[bank][:, :, :]).then_inc(
                    sem["dve_sem"], 1)
            dve_prog.append(d_pbf)
            cnt["dve_sem"] += 1
            pbf_done[b] = cnt["dve_sem"]

        # ================= table build =================
        build_items = [("n", q) for q in range(2, 64)] + \
                      [("t", q) for q in range(64, 128)]
        bank_owner = {}
        entry_done = {("n", 1): ("dve_sem", ident_done)}

        for j, (kind, q) in enumerate(build_items):
            bank = j % 8
            b = q & 1
            par = q >> 1

            waits = [("dve_sem", pbf_done[b]), entry_done[("n", par)]]
            if bank in bank_owner:
                waits.append(bank_owner[bank])
            elif bank in (2, 3):
                waits.append(("dve_sem", pbf_done[1]))

            def p_build(t, kind=kind, b=b, par=par, bank=bank, waits=tuple(waits)):
                for s_, c_ in waits:
                    t.wait_ge(sem[s_], c_)
                last = None
                for mc in range(2):
                    for kc in range(2):
                        if kind == "n":
                            lhsT = pbf[:, b, kc, mc * P:(mc + 1) * P]
                            rhs = rn_all[:, par - 1, kc, :]
                        else:
                            lhsT = rn_all[:, par - 1, kc, mc * P:(mc + 1) * P]
                            rhs = pbf[:, b, kc, :]
                        last = t.matmul(ps[bank][:, mc, :], lhsT, rhs,
                                        start=(kc == 0), stop=(kc == 1))
                last.then_inc(sem["pe_sem"], 1)
            pe_prog.append(p_build)
            cnt["pe_sem"] += 1

            ceng = "dve_sem" if j % 2 == 0 else "act_sem"
            prog = dve_prog if j % 2 == 0 else act_prog

            def x_copy(e, kind=kind, q=q, bank=bank, w=cnt["pe_sem"], ceng=ceng):
                e.wait_ge(sem["pe_sem"], w)
                dst = rn_all[:, q - 1, :, :] if kind == "n" \
                    else rt_all[:, q - 63, :, :]
                if ceng == "dve_sem":
                    e.tensor_copy(dst, ps[bank][:, :, :]).then_inc(sem[ceng], 1)
                else:
                    e.mul(dst, ps[bank][:, :, :], 1.0).then_inc(sem[ceng], 1)
            prog.append(x_copy)
            cnt[ceng] += 1
            entry_done[(kind, q)] = (ceng, cnt[ceng])
            bank_owner[bank] = (ceng, cnt[ceng])

        build_dve = cnt["dve_sem"]
        build_act = cnt["act_sem"]

        # ================= position phase =================
        # gpsimd: A-entry gather, one DMA per position
        def g_pos(g, bd=build_dve, ba=build_act):
            g.wait_ge(sem["in_sem"], IN_ALL)
            g.wait_ge(sem["dve_sem"], bd)
            g.wait_ge(sem["act_sem"], ba)
            regs = [g.alloc_register(f"ga{x}") for x in range(GB)]
            for kb in range(NGB):
                g.reg_load(regs, offs_a[kb:kb + 1, 0:GB])
                for j in range(GB):
                    i = GB * kb + j
                    if i >= NSA:
                        g.wait_ge(sem["mm1_sem"], i - NSA + 1)
                    src = bass.AP(rt_all, regs[j], [[TSTRIDE, P], [DIM, 2], [1, DIM]])
                    g.dma_start(stag_a[:, i % NSA, :, :], src).then_inc(
                        sem["stga_sem"], 16)
            for r in regs:
                g.free_register(r)
        gps_prog.append(g_pos)

        # sync: B stages (dedup'd) interleaved with batched output DMAs
        def s_pos(s, bd=build_dve, ba=build_act):
            s.wait_ge(sem["dve_sem"], bd)
            s.wait_ge(sem["act_sem"], ba)
            regs = [s.alloc_register(f"sb{x}") for x in range(GB)]
            events = ([("st", x, _blast(x, npos) - 12) for x in range(NSTAGE_B)]
                      + [("out", m, 4 * m + 20) for m in range(NGRP)])
            events.sort(key=lambda e: e[2])
            for kind, x, _ in events:
                if kind == "st":
                    if x % GB == 0:
                        s.reg_load(regs, offs_b[x // GB:x // GB + 1, 0:GB])
                    if x >= NSB:
                        s.wait_ge(sem["mm1_sem"], _blast(x - NSB, npos) + 1)
                    src = bass.AP(rn_all, regs[x % GB],
                                  [[NSTRIDE, P], [DIM, 2], [1, DIM]])
                    s.dma_start(stag_b[:, x % NSB, :, :], src).then_inc(
                        sem["stgb_sem"], 16)
                else:
                    m = x
                    s.wait_ge(sem["actp_sem"], 4 * (m + 1))
                    dst = bass.AP(out_ext, m * 4 * P * 2 * DIM,
                                  [[2 * DIM, P], [P * 2 * DIM, 4],
                                   [DIM, 2], [1, DIM]])
                    src = outb[:, (m % 2) * 4:(m % 2) * 4 + 4, :, :]
                    s.dma_start(dst, src).then_inc(sem[f"dma_s{m % 2}"], 16)
            s.wait_ge(sem["dma_s0"], 16 * ((NGRP + 1) // 2))
            s.wait_ge(sem["dma_s1"], 16 * (NGRP // 2))
            for r in regs:
                s.free_register(r)
        sync_prog.append(s_pos)

        def p_pos(t, bd=build_dve, ba=build_act):
            t.wait_ge(sem["in_sem"], IN_ALL)
            t.wait_ge(sem["dve_sem"], bd)
            t.wait_ge(sem["act_sem"], ba)
            scratch = [t.alloc_register(f"pc{x}") for x in range(GB)]
            rlo = t.alloc_register("rlo")
            rhi = t.alloc_register("rhi")
            for k in range(NB + 1):
                # row k of offs_c = packed C kc0-offset pairs for batch k-1
                t.reg_load(scratch, offs_c[k:k + 1, 0:GB])
                for g in range(4):
                    if k < NB:
                        i0 = BATCH * k + 4 * g
                        t.wait_ge(sem["stga_sem"], 16 * (i0 + 4))
                        t.wait_ge(sem["stgb_sem"],
                                  16 * (_bslot_stage(i0 + 3) + 1))
                        if i0 >= 4:
                            t.wait_ge(sem["dvex_sem"], i0)
                        for u in range(4):
                            i = i0 + u
                            last = None
                            for mc in range(2):
                                for kc in range(2):
                                    last = t.matmul(
                                        ps[i % 4][:, mc, :],
                                        stag_b[:, _bslot(i) % NSB, kc,
                                               mc * P:(mc + 1) * P],
                                        stag_a[:, i % NSA, kc, :],
                                        start=(kc == 0), stop=(kc == 1))
                            last.then_inc(sem["mm1_sem"], 1)
                    if k >= 1:
                        j0 = BATCH * (k - 1) + 4 * g
                        t.wait_ge(sem["dvex_sem"], j0 + 4)
                        if j0 >= 4:
                            t.wait_ge(sem["actp_sem"], j0)
                        for u in (0, 2):
                            w = scratch[(4 * g + u) // 2]
                            t.reg_alu(rlo, w, 65535, And)
                            t.reg_alu(rhi, w, 16, Shr)
                            v0 = t.snap(rlo)
                            v0d = t.snap(v0 + DIM)
                            v1 = t.snap(rhi)
                            v1d = t.snap(v1 + DIM)
                            for jj, r0, r1 in ((j0 + u, v0, v0d),
                                               (j0 + u + 1, v1, v1d)):
                                last = None
                                for mc in range(2):
                                    for kc in range(2):
                                        rhs = bass.AP(rn_all, r0 if kc == 0 else r1,
                                                      [[NSTRIDE, P], [1, DIM]])
                                        last = t.matmul(
                                            ps[4 + jj % 4][:, mc, :],
                                            stag_x[:, jj % NSX, kc,
                                                   mc * P:(mc + 1) * P],
                                            rhs, start=(kc == 0), stop=(kc == 1))
                                last.then_inc(sem["mm2_sem"], 1)
            for r in scratch + [rlo, rhi]:
                t.free_register(r)
        pe_prog.append(p_pos)

        def d_pos(d):
            for i in range(npos):
                d.wait_ge(sem["mm1_sem"], i + 1)
                if i >= NSX:
                    d.wait_ge(sem["mm2_sem"], i - NSX + 1)
                d.tensor_copy(stag_x[:, i % NSX, :, :],
                              ps[i % 4][:, :, :]).then_inc(sem["dvex_sem"], 1)
        dve_prog.append(d_pos)

        def a_pos(a):
            for i in range(npos):
                m = i // 4
                a.wait_ge(sem["mm2_sem"], i + 1)
                if i % 4 == 0 and m >= 2:
                    a.wait_ge(sem[f"dma_s{m % 2}"], 16 * (m // 2))
                a.mul(outb[:, i % NOUT, :, :],
                      ps[4 + i % 4][:, :, :], 1.0).then_inc(sem["actp_sem"], 1)
        act_prog.append(a_pos)

        if dump:
            def s_dump(s, bd=build_dve, ba=build_act):
                s.wait_ge(sem["dve_sem"], bd)
                s.wait_ge(sem["act_sem"], ba)
                s.dma_start(dbg_rn[:, :],
                            bass.AP(rn_all, 0, [[NSTRIDE, P], [1, NSTRIDE]])
                            ).then_inc(sem["in_sem"], 16)
                s.dma_start(dbg_rt[:, :],
                            bass.AP(rt_all, 0, [[TSTRIDE, P], [1, TSTRIDE]])
                            ).then_inc(sem["in_sem"], 16)
                s.dma_start(dbg_pbf[:, :],
                            bass.AP(pbf, 0, [[2 * 2 * DIM, P], [1, 2 * 2 * DIM]])
                            ).then_inc(sem["in_sem"], 16)
                s.wait_ge(sem["in_sem"], IN_ALL + 48)
            sync_prog.append(s_dump)

        # ================= emit =================
        with nc.Block() as block:
            @block.tensor
            def _(tensor):
                for fn in pe_prog:
                    fn(tensor)

            @block.vector
            def _(vector):
                for fn in dve_prog:
                    fn(vector)

            @block.scalar
            def _(scalar):
                for fn in act_prog:
                    fn(scalar)

            @block.gpsimd
            def _(gpsimd):
                for fn in gps_prog:
                    fn(gpsimd)

            @block.sync
            def _(sync):
                for fn in sync_prog:
                    fn(sync)

    return nc


def _bslot_stage(p):
    """static index of the B stage covering (sorted) position p"""
    return p // 2 if p < 2 * NPAIR_B else NPAIR_B + (p - 2 * NPAIR_B)


def _host_offsets(u):
    """u: (n,) int64 positions -> (idxA, idxB, idxC) table indices."""
    u = u.astype(np.int64)
    blen = np.zeros_like(u)
    t = u.copy()
    while np.any(t > 0):
        blen = np.where(t > 0, blen + 1, blen)
        t >>= 1
    k = blen - 1
    tA = np.minimum(k, 6)
    idxA = (1 << tA) + (u & ((1 << tA) - 1))
    tB = np.clip(k - 6, 0, 5)
    idxB = (1 << tB) + ((u >> 6) & ((1 << tB) - 1))
    tC = np.clip(k - 11, 0, 5)
    idxC = (1 << tC) + ((u >> 11) & ((1 << tC) - 1))
    short = u < 64
    idxA = np.where(short, 1, idxA)
    idxB = np.where(short, u, idxB)
    assert idxA.max() < 128 and idxB.max() < 64 and idxC.max() < 64
    assert np.all((idxA == 1) | (idxA >= 64))
    return idxA, idxB, idxC


def _schedule_core(idxB):
    """Sort a core's positions into 96 same-B pairs + singles.

    Returns (order, stage_b_idx): order[i] = original position index of the
    i-th scheduled position; stage_b_idx[s] = idxB staged by stage s."""
    npos = len(idxB)
    npair = NPAIR_B
    by_val = {}
    for p in np.argsort(idxB, kind="stable"):
        by_val.setdefault(int(idxB[p]), []).append(int(p))
    pairs, singles = [], []
    for v, plist in by_val.items():
        while len(plist) >= 2:
            pairs.append((plist.pop(), plist.pop()))
        singles.extend(plist)
    assert len(pairs) >= npair, (len(pairs), npair)
    while len(pairs) > npair:
        a, b = pairs.pop()
        singles.extend([a, b])
    assert 2 * len(pairs) + len(singles) == npos
    order = [p for pr in pairs for p in pr] + singles
    stage_b = [int(idxB[pr[0]]) for pr in pairs] + \
              [int(idxB[p]) for p in singles]
    return np.array(order), np.array(stage_b)


def kernel(primitives, identity, unique):
    global LAST_RESULTS
    from concourse.bass_utils import run_bass_kernel_spmd

    prims = np.ascontiguousarray(np.asarray(primitives, dtype=np.float32))
    primsT = np.ascontiguousarray(prims.transpose(0, 2, 1))
    u = np.asarray(unique).astype(np.int64).ravel()
    n = u.shape[0]
    assert n % NCORES == 0
    npos = n // NCORES
    NB = npos // BATCH
    NGB = npos // GB
    NSTAGE_B = NPAIR_B + (npos - 2 * NPAIR_B)

    idxA, idxB, idxC = _host_offsets(u)

    if npos not in _NC_CACHE:
        nc = _build_nc(npos)
        nc.compile()
        _NC_CACHE[npos] = nc
    nc = _NC_CACHE[npos]

    in_maps = []
    orders = []
    for c in range(NCORES):
        sl = slice(c * npos, (c + 1) * npos)
        order, stage_b = _schedule_core(idxB[sl])
        orders.append(order)
        a_sorted = idxA[sl][order]
        c_sorted = idxC[sl][order]
        slotA = np.where(a_sorted == 1, 0, a_sorted - 63)
        oA = (slotA * ENT).astype(np.int32)
        oB = ((stage_b - 1) * ENT).astype(np.int32)
        oC = ((c_sorted - 1) * ENT).astype(np.int32)  # kc0 element offsets

        arr_a = np.zeros((P, GB), np.int32)
        arr_a[:NGB, :] = oA.reshape(NGB, GB)
        arr_b = np.zeros((P, GB), np.int32)
        nbrow = (NSTAGE_B + GB - 1) // GB
        obp = np.zeros(nbrow * GB, np.int32)
        obp[:NSTAGE_B] = oB
        arr_b[:nbrow, :] = obp.reshape(nbrow, GB)
        arr_c = np.zeros((P, GB), np.int32)
        w = (oC[0::2].astype(np.int64)
             | (oC[1::2].astype(np.int64) << 16)).astype(np.int32)
        arr_c[1:NB + 1, :] = w.reshape(NB, GB)
        in_maps.append({"prims": prims, "primsT": primsT,
                        "offs_a": arr_a, "offs_b": arr_b, "offs_c": arr_c})

    import os
    trace_dir = os.environ.get("KERNEL_TRACE_DIR")
    res = run_bass_kernel_spmd(nc, in_maps, core_ids=list(range(NCORES)),
                               tmpdir=trace_dir)
    LAST_RESULTS = res

    out = np.empty((n, DIM, DIM), np.float32)
    for c in range(NCORES):
        o = np.asarray(res.results[c]["out"])  # (npos, P, 2, DIM) f16
        o = o.transpose(0, 2, 1, 3).reshape(npos, DIM, DIM).astype(np.float32)
        inv = np.empty(npos, np.int64)
        inv[orders[c]] = np.arange(npos)
        out[c * npos:(c + 1) * npos] = o[inv]

    ident = np.asarray(identity, dtype=np.float32)[0]
    if not np.allclose(ident, np.eye(DIM, dtype=np.float32)):
        out = np.einsum("ij,njk->nik", ident, out).astype(np.float32)
    return out


# revision 17
# speedup vs baseline: 1.3550x; 1.0092x over previous
"""Trainium2 Bass kernel for nn_BinaryPathEncoder (v3).

Math: for each position p, R(p) = M_{b0} @ M_{b1} @ ... (LSB-first binary
path, leading 1 stripped), M_b = expm(B_b - B_b^T)^T.  Split the <=16-step
path 6+5+5 so R(p) = R_A @ R_B @ R_C with f16 table lookups:
  mm1: X1T = (R_A R_B)^T = matmul(lhsT=stag_b, rhs=stag_a)     [all static]
  mm2: O   = R_A R_B R_C  = matmul(lhsT=stag_x, rhs=Rn[idxC])  [snap-value AP]
expm on device: f32r Taylor-Horner (S=2/N=5) with pre-scaled A/k buffers,
four interleaved chains, transpose-free squarings.

Position-phase design (per core, npos=256):
- mm1 operands are DMA-staged: A entries gathered per position (gpsimd,
  register-offset src APs, batched reg loads), B entries staged by the
  sync queue with host-side dedup: positions are sorted by idxB and
  arranged as 96 same-B pairs + 64 singles (always feasible), so only
  160 B stage DMAs.  The PE's mm1 instructions are fully static ->
  109ns/matmul streaming.
- mm2's moving operand reads the natural table via snap()-materialized
  ScalarValue AP offsets (walrus encodes those inline in $R[8]/$R[9] with
  no per-matmul prefix, unlike raw RegisterHandle APs).  Two 16-bit
  offsets per word, +DIM values derived with snap adds.
- Output: [npos, P, 2, DIM] layout (1KB contiguous runs per partition),
  one dma_start per 4 positions.
"""

import contextlib
import numpy as np

DIM = 256
NCORES = 8
P = 128

NAT_E = 63                     # natural table entries (q in [1,64))
TRA_E = 65                     # transposed: slot0=identity, 1..64 = q in [64,128)
ENT = 512                      # entry elements per partition (2 kc x 256)
NSTRIDE = NAT_E * ENT          # per-partition stride of rn_all
TSTRIDE = TRA_E * ENT          # per-partition stride of rt_all

BATCH = 16                     # PE batch (one C-offset reg_load per batch)
GB = 8                         # gather reg_load batch
NSA = 24                       # stag_a ring slots
NSB = 16                       # stag_b ring slots
NSX = 24                       # stag_x ring slots
NOUT = 8                       # outb slots (2 dma groups of 4)
NPAIR_B = 96                   # same-B pairs (2 positions / stage)
EXPM_S = 2
EXPM_N = 5

_NC_CACHE = {}
LAST_RESULTS = None


def _bslot(p):
    """static stag_b slot for (sorted) position p"""
    s = p // 2 if p < 2 * NPAIR_B else NPAIR_B + (p - 2 * NPAIR_B)
    return s


def _blast(s, npos):
    """static last position covered by B stage s"""
    return 2 * s + 1 if s < NPAIR_B else 2 * NPAIR_B + (s - NPAIR_B)


def _build_nc(npos, debug=False, dump=False):
    from concourse import bass, bacc, mybir

    f32 = mybir.dt.float32
    f32r = mybir.dt.float32r
    f16 = mybir.dt.float16
    i32 = mybir.dt.int32
    Sub = mybir.AluOpType.subtract
    Add = mybir.AluOpType.add
    Eq = mybir.AluOpType.is_equal
    And = mybir.AluOpType.bitwise_and
    Shr = mybir.AluOpType.logical_shift_right

    assert npos % BATCH == 0
    NB = npos // BATCH
    NGB = npos // GB
    NGRP = npos // 4
    NSTAGE_B = NPAIR_B + (npos - 2 * NPAIR_B)   # 160 for npos=256
    assert npos - 2 * NPAIR_B >= 0

    nc = bacc.Bacc("TRN2", target_bir_lowering=False, debug=debug)

    prims_ext = nc.dram_tensor("prims", [2, DIM, DIM], f32, kind="ExternalInput")
    primsT_ext = nc.dram_tensor("primsT", [2, DIM, DIM], f32, kind="ExternalInput")
    offs_a_ext = nc.dram_tensor("offs_a", [P, GB], i32, kind="ExternalInput")
    offs_b_ext = nc.dram_tensor("offs_b", [P, GB], i32, kind="ExternalInput")
    offs_c_ext = nc.dram_tensor("offs_c", [P, GB], i32, kind="ExternalInput")
    out_ext = nc.dram_tensor("out", [npos, P, 2, DIM], f16, kind="ExternalOutput")
    if dump:
        dbg_rn = nc.dram_tensor("dbg_rn", [P, NSTRIDE], f16, kind="ExternalOutput")
        dbg_rt = nc.dram_tensor("dbg_rt", [P, TSTRIDE], f16, kind="ExternalOutput")
        dbg_pbf = nc.dram_tensor("dbg_pbf", [P, 2 * 2 * DIM], f16,
                                 kind="ExternalOutput")

    inv2s = 1.0 / (2.0 ** EXPM_S)

    with contextlib.ExitStack() as ctx:
        sem = {}
        for name in ["in_sem", "gps_sem", "pe_sem", "dve_sem", "act_sem",
                     "mm1_sem", "mm2_sem", "dvex_sem", "actp_sem",
                     "stga_sem", "stgb_sem", "dma_s0", "dma_s1"]:
            sem[name] = ctx.enter_context(nc.semaphore(name))

        # ---- persistent SBUF ----
        rt_all = ctx.enter_context(nc.sbuf_tensor("rt_all", [P, TRA_E, 2, DIM], f16))
        rn_all = ctx.enter_context(nc.sbuf_tensor("rn_all", [P, NAT_E, 2, DIM], f16))
        pbf = ctx.enter_context(nc.sbuf_tensor("pbf", [P, 2, 2, DIM], f16))
        offs_a = ctx.enter_context(nc.sbuf_tensor("offs_a_sb", [P, GB], i32))
        offs_b = ctx.enter_context(nc.sbuf_tensor("offs_b_sb", [P, GB], i32))
        offs_c = ctx.enter_context(nc.sbuf_tensor("offs_c_sb", [P, GB], i32))

        # ---- expm-phase buffers; space reused by position-phase staging ----
        tmp_ctx = contextlib.ExitStack()
        prim = [tmp_ctx.enter_context(nc.sbuf_tensor(f"prim{b}", [P, 2, DIM], f32))
                for b in range(2)]
        primT = [tmp_ctx.enter_context(nc.sbuf_tensor(f"primT{b}", [P, 2, DIM], f32))
                 for b in range(2)]
        tmp = [tmp_ctx.enter_context(nc.sbuf_tensor(f"tmp{b}", [P, 2, DIM], f32r))
               for b in range(2)]
        ank = [[None] + [tmp_ctx.enter_context(
                    nc.sbuf_tensor(f"an{k}_{b}", [P, 2, DIM], f32r))
                for k in range(1, 5)] for b in range(2)]
        apk = [[None] + [tmp_ctx.enter_context(
                    nc.sbuf_tensor(f"ap{k}_{b}", [P, 2, DIM], f32r))
                for k in range(1, 5)] for b in range(2)]
        apiE = [tmp_ctx.enter_context(nc.sbuf_tensor(f"apiE{b}", [P, 2, DIM], f32r))
                for b in range(2)]
        apiT = [tmp_ctx.enter_context(nc.sbuf_tensor(f"apiT{b}", [P, 2, DIM], f32r))
                for b in range(2)]
        yE = [tmp_ctx.enter_context(nc.sbuf_tensor(f"yE{b}", [P, 2, DIM], f32r))
              for b in range(2)]
        yT = [tmp_ctx.enter_context(nc.sbuf_tensor(f"yT{b}", [P, 2, DIM], f32r))
              for b in range(2)]
        eE = [tmp_ctx.enter_context(nc.sbuf_tensor(f"eE{b}", [P, 2, DIM], f32r))
              for b in range(2)]
        eT = [tmp_ctx.enter_context(nc.sbuf_tensor(f"eT{b}", [P, 2, DIM], f32r))
              for b in range(2)]
        ci = tmp_ctx.enter_context(nc.sbuf_tensor("ci", [P, DIM], f32))
        pi = tmp_ctx.enter_context(nc.sbuf_tensor("pi", [P, 2], f32))
        identf = tmp_ctx.enter_context(nc.sbuf_tensor("identf", [P, 2, DIM], f32))
        tmp_ctx.close()
        stag_a = ctx.enter_context(nc.sbuf_tensor("stag_a", [P, NSA, 2, DIM], f16))
        stag_b = ctx.enter_context(nc.sbuf_tensor("stag_b", [P, NSB, 2, DIM], f16))
        stag_x = ctx.enter_context(nc.sbuf_tensor("stag_x", [P, NSX, 2, DIM], f16))
        outb = ctx.enter_context(nc.sbuf_tensor("outb", [P, NOUT, 2, DIM], f16))
        ps = [ctx.enter_context(nc.psum_tensor(f"ps{j}", [P, 2, DIM], f32))
              for j in range(8)]

        cnt = {k: 0 for k in sem}
        pe_prog, dve_prog, act_prog, gps_prog, sync_prog = [], [], [], [], []

        # ================= input DMA (sync) =================
        def s_in(s):
            for b in range(2):
                for kc in range(2):
                    s.dma_start(prim[b][:, kc, :],
                                prims_ext[b, kc * P:(kc + 1) * P, :]
                                ).then_inc(sem["in_sem"], 16)
                    s.dma_start(primT[b][:, kc, :],
                                primsT_ext[b, kc * P:(kc + 1) * P, :]
                                ).then_inc(sem["in_sem"], 16)
            s.dma_start(offs_a[:, :], offs_a_ext[:, :]).then_inc(sem["in_sem"], 16)
            s.dma_start(offs_b[:, :], offs_b_ext[:, :]).then_inc(sem["in_sem"], 16)
            s.dma_start(offs_c[:, :], offs_c_ext[:, :]).then_inc(sem["in_sem"], 16)
        sync_prog.append(s_in)
        cnt["in_sem"] = 16 * 11
        IN_ALL = cnt["in_sem"]
        IN_PRIM = {0: 16 * 4, 1: 16 * 8}

        # ================= identity (gpsimd iota + DVE eq) =================
        def g_iota(g):
            g.iota(ci[:, :], [[1, DIM]], channel_multiplier=0,
                   allow_small_or_imprecise_dtypes=True)
            g.iota(pi[:, 0:1], [[1, 1]], channel_multiplier=1,
                   allow_small_or_imprecise_dtypes=True)
            g.iota(pi[:, 1:2], [[1, 1]], base=P, channel_multiplier=1,
                   allow_small_or_imprecise_dtypes=True).then_inc(sem["gps_sem"], 1)
        gps_prog.append(g_iota)
        cnt["gps_sem"] += 1

        def d_ident(d, w=cnt["gps_sem"]):
            d.wait_ge(sem["gps_sem"], w)
            for kc in range(2):
                d.tensor_tensor(out=identf[:, kc, :], in0=ci[:, :],
                                in1=pi[:, kc:kc + 1].to_broadcast([P, DIM]), op=Eq)
            d.drain()
            d.tensor_copy(rn_all[:, 0, :, :], identf[:, :, :])
            d.tensor_copy(rt_all[:, 0, :, :], identf[:, :, :]).then_inc(
                sem["dve_sem"], 1)
        dve_prog.append(d_ident)
        cnt["dve_sem"] += 1
        ident_done = cnt["dve_sem"]

        # ================= expm: 4 interleaved chains =================
        dve_ready = {}

        def d_pre(b):
            # NOTE: wait for ALL input DMAs -- completions of distinct
            # dma_starts are NOT ordered (packets spread across 16 engines),
            # so a partial in_sem count does not identify WHICH DMAs landed.
            def fn(d, b=b, win=IN_ALL):
                d.wait_ge(sem["in_sem"], win)
                d.tensor_tensor(out=tmp[b][:, :, :], in0=prim[b][:, :, :],
                                in1=primT[b][:, :, :], op=Sub)
                d.drain()
                d.tensor_scalar_mul(yE[b][:, :, :], tmp[b][:, :, :], inv2s / EXPM_N)
                d.tensor_scalar_mul(yT[b][:, :, :], tmp[b][:, :, :], -inv2s / EXPM_N)
                d.tensor_scalar_mul(ank[b][4][:, :, :], tmp[b][:, :, :], -inv2s / 4)
                d.tensor_scalar_mul(apk[b][4][:, :, :], tmp[b][:, :, :],
                                    inv2s / 4).then_inc(sem["dve_sem"], 1)
            return fn

        for b in range(2):
            dve_prog.append(d_pre(b))
            cnt["dve_sem"] += 1
            dve_ready[(b, "r0")] = cnt["dve_sem"]

        def d_pre2(b):
            def fn(d, b=b):
                for k in (3, 2, 1):
                    d.tensor_scalar_mul(ank[b][k][:, :, :], tmp[b][:, :, :],
                                        -inv2s / k)
                    d.tensor_scalar_mul(apk[b][k][:, :, :], tmp[b][:, :, :],
                                        inv2s / k)
                d.drain()
                d.tensor_tensor(out=apiE[b][:, :, :], in0=apk[b][1][:, :, :],
                                in1=identf[:, :, :], op=Add)
                d.tensor_tensor(out=apiT[b][:, :, :], in0=ank[b][1][:, :, :],
                                in1=identf[:, :, :],
                                op=Add).then_inc(sem["dve_sem"], 1)
            return fn

        for b in range(2):
            dve_prog.append(d_pre2(b))
            cnt["dve_sem"] += 1
            dve_ready[(b, "pre2")] = cnt["dve_sem"]

        chains = [(0, "E"), (0, "T"), (1, "E"), (1, "T")]
        chain_ps = {c: i for i, c in enumerate(chains)}

        for r in range(4):
            k = 4 - r
            for c in chains:
                b, ch = c
                if r == 0:
                    wd = dve_ready[(b, "r0")]
                elif r == 3:
                    wd = max(dve_ready[(b, "pre2")], dve_ready[(b, c, r - 1)])
                else:
                    wd = max(dve_ready[(b, "pre2")], dve_ready[(b, c, r - 1)])

                def p_h(t, b=b, ch=ch, k=k, wd=wd, bank=chain_ps[(b, ch)]):
                    t.wait_ge(sem["dve_sem"], wd)
                    lhs = ank[b][k] if ch == "E" else apk[b][k]
                    y = yE[b] if ch == "E" else yT[b]
                    last = None
                    for mc in range(2):
                        for kc in range(2):
                            last = t.matmul(ps[bank][:, mc, :],
                                            lhs[:, kc, mc * P:(mc + 1) * P],
                                            y[:, kc, :],
                                            start=(kc == 0), stop=(kc == 1))
                    last.then_inc(sem["pe_sem"], 1)
                pe_prog.append(p_h)
                cnt["pe_sem"] += 1

                if r < 3:
                    def d_h(d, b=b, ch=ch, k=k, wp=cnt["pe_sem"],
                            bank=chain_ps[(b, ch)]):
                        d.wait_ge(sem["pe_sem"], wp)
                        y = yE[b] if ch == "E" else yT[b]
                        d.tensor_tensor(out=y[:, :, :], in0=ps[bank][:, :, :],
                                        in1=apk[b][k][:, :, :],
                                        op=(Add if ch == "E" else Sub)
                                        ).then_inc(sem["dve_sem"], 1)
                else:
                    def d_h(d, b=b, ch=ch, wp=cnt["pe_sem"],
                            bank=chain_ps[(b, ch)]):
                        d.wait_ge(sem["pe_sem"], wp)
                        e = eE[b] if ch == "E" else eT[b]
                        api = apiE[b] if ch == "E" else apiT[b]
                        d.tensor_tensor(out=e[:, :, :], in0=ps[bank][:, :, :],
                                        in1=api[:, :, :],
                                        op=Add).then_inc(sem["dve_sem"], 1)
                dve_prog.append(d_h)
                cnt["dve_sem"] += 1
                dve_ready[(b, c, r)] = cnt["dve_sem"]

        # sq1 (both chains), out -> y buffers (free after Horner)
        for c in chains:
            b, ch = c
            wd = max(dve_ready[(b, (b, "E"), 3)], dve_ready[(b, (b, "T"), 3)])

            def p_sq1(t, b=b, ch=ch, wd=wd, bank=chain_ps[(b, ch)]):
                t.wait_ge(sem["dve_sem"], wd)
                lhs = eT[b] if ch == "E" else eE[b]
                rhs = eE[b] if ch == "E" else eT[b]
                last = None
                for mc in range(2):
                    for kc in range(2):
                        last = t.matmul(ps[bank][:, mc, :],
                                        lhs[:, kc, mc * P:(mc + 1) * P],
                                        rhs[:, kc, :],
                                        start=(kc == 0), stop=(kc == 1))
                last.then_inc(sem["pe_sem"], 1)
            pe_prog.append(p_sq1)
            cnt["pe_sem"] += 1

            def d_sq1(d, b=b, ch=ch, wp=cnt["pe_sem"], bank=chain_ps[(b, ch)]):
                d.wait_ge(sem["pe_sem"], wp)
                dst = yE[b] if ch == "E" else yT[b]
                d.tensor_copy(dst[:, :, :], ps[bank][:, :, :]).then_inc(
                    sem["dve_sem"], 1)
            dve_prog.append(d_sq1)
            cnt["dve_sem"] += 1
            dve_ready[(b, c, "sq1")] = cnt["dve_sem"]

        # sq2 (E chains only): P_b -> pbf f16 straight from PSUM
        pbf_done = {}
        for b in range(2):
            wd = max(dve_ready[(b, (b, "E"), "sq1")],
                     dve_ready[(b, (b, "T"), "sq1")])

            def p_sq2(t, b=b, wd=wd, bank=chain_ps[(b, "E")]):
                t.wait_ge(sem["dve_sem"], wd)
                last = None
                for mc in range(2):
                    for kc in range(2):
                        last = t.matmul(ps[bank][:, mc, :],
                                        yT[b][:, kc, mc * P:(mc + 1) * P],
                                        yE[b][:, kc, :],
                                        start=(kc == 0), stop=(kc == 1))
                last.then_inc(sem["pe_sem"], 1)
            pe_prog.append(p_sq2)
            cnt["pe_sem"] += 1

            def d_pbf(d, b=b, wp=cnt["pe_sem"], bank=chain_ps[(b, "E")]):
                d.wait_ge(sem["pe_sem"], wp)
                d.tensor_copy(pbf[:, b, :, :], ps[bank][:, :, :]).then_inc(
                    sem["dve_sem"], 1)
            dve_prog.append(d_pbf)
            cnt["dve_sem"] += 1
            pbf_done[b] = cnt["dve_sem"]

        # ================= table build =================
        build_items = [("n", q) for q in range(2, 64)] + \
                      [("t", q) for q in range(64, 128)]
        bank_owner = {}
        entry_done = {("n", 1): ("dve_sem", ident_done)}

        for j, (kind, q) in enumerate(build_items):
            bank = j % 8
            b = q & 1
            par = q >> 1

            waits = [("dve_sem", pbf_done[b]), entry_done[("n", par)]]
            if bank in bank_owner:
                waits.append(bank_owner[bank])
            elif bank in (2, 3):
                waits.append(("dve_sem", pbf_done[1]))

            def p_build(t, kind=kind, b=b, par=par, bank=bank, waits=tuple(waits)):
                for s_, c_ in waits:
                    t.wait_ge(sem[s_], c_)
                last = None
                for mc in range(2):
                    for kc in range(2):
                        if kind == "n":
                            lhsT = pbf[:, b, kc, mc * P:(mc + 1) * P]
                            rhs = rn_all[:, par - 1, kc, :]
                        else:
                            lhsT = rn_all[:, par - 1, kc, mc * P:(mc + 1) * P]
                            rhs = pbf[:, b, kc, :]
                        last = t.matmul(ps[bank][:, mc, :], lhsT, rhs,
                                        start=(kc == 0), stop=(kc == 1))
                last.then_inc(sem["pe_sem"], 1)
            pe_prog.append(p_build)
            cnt["pe_sem"] += 1

            ceng = "dve_sem" if j % 2 == 0 else "act_sem"
            prog = dve_prog if j % 2 == 0 else act_prog

            def x_copy(e, kind=kind, q=q, bank=bank, w=cnt["pe_sem"], ceng=ceng):
                e.wait_ge(sem["pe_sem"], w)
                dst = rn_all[:, q - 1, :, :] if kind == "n" \
                    else rt_all[:, q - 63, :, :]
                if ceng == "dve_sem":
                    e.tensor_copy(dst, ps[bank][:, :, :]).then_inc(sem[ceng], 1)
                else:
                    e.mul(dst, ps[bank][:, :, :], 1.0).then_inc(sem[ceng], 1)
            prog.append(x_copy)
            cnt[ceng] += 1
            entry_done[(kind, q)] = (ceng, cnt[ceng])
            bank_owner[bank] = (ceng, cnt[ceng])

        build_dve = cnt["dve_sem"]
        build_act = cnt["act_sem"]

        # ================= position phase =================
        # gpsimd: A-entry gather, one DMA per position
        def g_pos(g, bd=build_dve, ba=build_act):
            g.wait_ge(sem["in_sem"], IN_ALL)
            g.wait_ge(sem["dve_sem"], bd)
            g.wait_ge(sem["act_sem"], ba)
            regs = [g.alloc_register(f"ga{x}") for x in range(GB)]
            for kb in range(NGB):
                g.reg_load(regs, offs_a[kb:kb + 1, 0:GB])
                for j in range(GB):
                    i = GB * kb + j
                    if i >= NSA:
                        g.wait_ge(sem["mm1_sem"], i - NSA + 1)
                    src = bass.AP(rt_all, regs[j], [[TSTRIDE, P], [DIM, 2], [1, DIM]])
                    g.dma_start(stag_a[:, i % NSA, :, :], src).then_inc(
                        sem["stga_sem"], 16)
            for r in regs:
                g.free_register(r)
        gps_prog.append(g_pos)

        # sync: B stages (dedup'd) interleaved with batched output DMAs
        def s_pos(s, bd=build_dve, ba=build_act):
            s.wait_ge(sem["dve_sem"], bd)
            s.wait_ge(sem["act_sem"], ba)
            regs = [s.alloc_register(f"sb{x}") for x in range(GB)]
            events = ([("st", x, _blast(x, npos) - 12) for x in range(NSTAGE_B)]
                      + [("out", m, 4 * m + 20) for m in range(NGRP)])
            events.sort(key=lambda e: e[2])
            for kind, x, _ in events:
                if kind == "st":
                    if x % GB == 0:
                        if x > 0:
                            # HWDGE expands descriptors asynchronously: the
                            # previous batch's stage DMAs must have completed
                            # (=> descriptors consumed) before their registers
                            # are overwritten.
                            s.wait_ge(sem["stgb_sem"], 16 * x)
                        s.reg_load(regs, offs_b[x // GB:x // GB + 1, 0:GB])
                    if x >= NSB:
                        s.wait_ge(sem["mm1_sem"], _blast(x - NSB, npos) + 1)
                    src = bass.AP(rn_all, regs[x % GB],
                                  [[NSTRIDE, P], [DIM, 2], [1, DIM]])
                    s.dma_start(stag_b[:, x % NSB, :, :], src).then_inc(
                        sem["stgb_sem"], 16)
                else:
                    m = x
                    s.wait_ge(sem["actp_sem"], 4 * (m + 1))
                    dst = bass.AP(out_ext, m * 4 * P * 2 * DIM,
                                  [[2 * DIM, P], [P * 2 * DIM, 4],
                                   [DIM, 2], [1, DIM]])
                    src = outb[:, (m % 2) * 4:(m % 2) * 4 + 4, :, :]
                    s.dma_start(dst, src).then_inc(sem[f"dma_s{m % 2}"], 16)
            s.wait_ge(sem["dma_s0"], 16 * ((NGRP + 1) // 2))
            s.wait_ge(sem["dma_s1"], 16 * (NGRP // 2))
            for r in regs:
                s.free_register(r)
        sync_prog.append(s_pos)

        def p_pos(t, bd=build_dve, ba=build_act):
            t.wait_ge(sem["in_sem"], IN_ALL)
            t.wait_ge(sem["dve_sem"], bd)
            t.wait_ge(sem["act_sem"], ba)
            scratch = [t.alloc_register(f"pc{x}") for x in range(GB)]
            rlo = t.alloc_register("rlo")
            rhi = t.alloc_register("rhi")
            for k in range(NB + 1):
                # row k of offs_c = packed C kc0-offset pairs for batch k-1
                t.reg_load(scratch, offs_c[k:k + 1, 0:GB])
                for g in range(4):
                    if k < NB:
                        i0 = BATCH * k + 4 * g
                        t.wait_ge(sem["stga_sem"], 16 * (i0 + 4))
                        t.wait_ge(sem["stgb_sem"],
                                  16 * (_bslot_stage(i0 + 3) + 1))
                        if i0 >= 4:
                            t.wait_ge(sem["dvex_sem"], i0)
                        for u in range(4):
                            i = i0 + u
                            last = None
                            for mc in range(2):
                                for kc in range(2):
                                    last = t.matmul(
                                        ps[i % 4][:, mc, :],
                                        stag_b[:, _bslot(i) % NSB, kc,
                                               mc * P:(mc + 1) * P],
                                        stag_a[:, i % NSA, kc, :],
                                        start=(kc == 0), stop=(kc == 1))
                            last.then_inc(sem["mm1_sem"], 1)
                    if k >= 1:
                        j0 = BATCH * (k - 1) + 4 * g
                        t.wait_ge(sem["dvex_sem"], j0 + 4)
                        if j0 >= 4:
                            t.wait_ge(sem["actp_sem"], j0)
                        for u in (0, 2):
                            w = scratch[(4 * g + u) // 2]
                            t.reg_alu(rlo, w, 65535, And)
                            t.reg_alu(rhi, w, 16, Shr)
                            v0 = t.snap(rlo)
                            v0d = t.snap(v0 + DIM)
                            v1 = t.snap(rhi)
                            v1d = t.snap(v1 + DIM)
                            for jj, r0, r1 in ((j0 + u, v0, v0d),
                                               (j0 + u + 1, v1, v1d)):
                                last = None
                                for mc in range(2):
                                    for kc in range(2):
                                        rhs = bass.AP(rn_all, r0 if kc == 0 else r1,
                                                      [[NSTRIDE, P], [1, DIM]])
                                        last = t.matmul(
                                            ps[4 + jj % 4][:, mc, :],
                                            stag_x[:, jj % NSX, kc,
                                                   mc * P:(mc + 1) * P],
                                            rhs, start=(kc == 0), stop=(kc == 1))
                                last.then_inc(sem["mm2_sem"], 1)
            for r in scratch + [rlo, rhi]:
                t.free_register(r)
        pe_prog.append(p_pos)

        def d_pos(d):
            for i in range(npos):
                d.wait_ge(sem["mm1_sem"], i + 1)
                if i >= NSX:
                    d.wait_ge(sem["mm2_sem"], i - NSX + 1)
                d.tensor_copy(stag_x[:, i % NSX, :, :],
                              ps[i % 4][:, :, :]).then_inc(sem["dvex_sem"], 1)
        dve_prog.append(d_pos)

        def a_pos(a):
            for i in range(npos):
                m = i // 4
                a.wait_ge(sem["mm2_sem"], i + 1)
                if i % 4 == 0 and m >= 2:
                    a.wait_ge(sem[f"dma_s{m % 2}"], 16 * (m // 2))
                a.mul(outb[:, i % NOUT, :, :],
                      ps[4 + i % 4][:, :, :], 1.0).then_inc(sem["actp_sem"], 1)
        act_prog.append(a_pos)

        if dump:
            def s_dump(s, bd=build_dve, ba=build_act):
                s.wait_ge(sem["dve_sem"], bd)
                s.wait_ge(sem["act_sem"], ba)
                s.dma_start(dbg_rn[:, :],
                            bass.AP(rn_all, 0, [[NSTRIDE, P], [1, NSTRIDE]])
                            ).then_inc(sem["in_sem"], 16)
                s.dma_start(dbg_rt[:, :],
                            bass.AP(rt_all, 0, [[TSTRIDE, P], [1, TSTRIDE]])
                            ).then_inc(sem["in_sem"], 16)
                s.dma_start(dbg_pbf[:, :],
                            bass.AP(pbf, 0, [[2 * 2 * DIM, P], [1, 2 * 2 * DIM]])
                            ).then_inc(sem["in_sem"], 16)
                s.wait_ge(sem["in_sem"], IN_ALL + 48)
            sync_prog.append(s_dump)

        # ================= emit =================
        with nc.Block() as block:
            @block.tensor
            def _(tensor):
                for fn in pe_prog:
                    fn(tensor)

            @block.vector
            def _(vector):
                for fn in dve_prog:
                    fn(vector)

            @block.scalar
            def _(scalar):
                for fn in act_prog:
                    fn(scalar)

            @block.gpsimd
            def _(gpsimd):
                for fn in gps_prog:
                    fn(gpsimd)

            @block.sync
            def _(sync):
                for fn in sync_prog:
                    fn(sync)

    return nc


def _bslot_stage(p):
    """static index of the B stage covering (sorted) position p"""
    return p // 2 if p < 2 * NPAIR_B else NPAIR_B + (p - 2 * NPAIR_B)


def _host_offsets(u):
    """u: (n,) int64 positions -> (idxA, idxB, idxC) table indices."""
    u = u.astype(np.int64)
    blen = np.zeros_like(u)
    t = u.copy()
    while np.any(t > 0):
        blen = np.where(t > 0, blen + 1, blen)
        t >>= 1
    k = blen - 1
    tA = np.minimum(k, 6)
    idxA = (1 << tA) + (u & ((1 << tA) - 1))
    tB = np.clip(k - 6, 0, 5)
    idxB = (1 << tB) + ((u >> 6) & ((1 << tB) - 1))
    tC = np.clip(k - 11, 0, 5)
    idxC = (1 << tC) + ((u >> 11) & ((1 << tC) - 1))
    short = u < 64
    idxA = np.where(short, 1, idxA)
    idxB = np.where(short, u, idxB)
    assert idxA.max() < 128 and idxB.max() < 64 and idxC.max() < 64
    assert np.all((idxA == 1) | (idxA >= 64))
    return idxA, idxB, idxC


def _schedule_core(idxB):
    """Sort a core's positions into 96 same-B pairs + singles.

    Returns (order, stage_b_idx): order[i] = original position index of the
    i-th scheduled position; stage_b_idx[s] = idxB staged by stage s."""
    npos = len(idxB)
    npair = NPAIR_B
    by_val = {}
    for p in np.argsort(idxB, kind="stable"):
        by_val.setdefault(int(idxB[p]), []).append(int(p))
    pairs, singles = [], []
    for v, plist in by_val.items():
        while len(plist) >= 2:
            pairs.append((plist.pop(), plist.pop()))
        singles.extend(plist)
    assert len(pairs) >= npair, (len(pairs), npair)
    while len(pairs) > npair:
        a, b = pairs.pop()
        singles.extend([a, b])
    assert 2 * len(pairs) + len(singles) == npos
    order = [p for pr in pairs for p in pr] + singles
    stage_b = [int(idxB[pr[0]]) for pr in pairs] + \
              [int(idxB[p]) for p in singles]
    return np.array(order), np.array(stage_b)


def kernel(primitives, identity, unique):
    global LAST_RESULTS
    from concourse.bass_utils import run_bass_kernel_spmd

    prims = np.ascontiguousarray(np.asarray(primitives, dtype=np.float32))
    primsT = np.ascontiguousarray(prims.transpose(0, 2, 1))
    u = np.asarray(unique).astype(np.int64).ravel()
    n = u.shape[0]
    assert n % NCORES == 0
    npos = n // NCORES
    NB = npos // BATCH
    NGB = npos // GB
    NSTAGE_B = NPAIR_B + (npos - 2 * NPAIR_B)

    idxA, idxB, idxC = _host_offsets(u)

    if npos not in _NC_CACHE:
        nc = _build_nc(npos)
        nc.compile()
        _NC_CACHE[npos] = nc
    nc = _NC_CACHE[npos]

    in_maps = []
    orders = []
    for c in range(NCORES):
        sl = slice(c * npos, (c + 1) * npos)
        order, stage_b = _schedule_core(idxB[sl])
        orders.append(order)
        a_sorted = idxA[sl][order]
        c_sorted = idxC[sl][order]
        slotA = np.where(a_sorted == 1, 0, a_sorted - 63)
        oA = (slotA * ENT).astype(np.int32)
        oB = ((stage_b - 1) * ENT).astype(np.int32)
        oC = ((c_sorted - 1) * ENT).astype(np.int32)  # kc0 element offsets

        arr_a = np.zeros((P, GB), np.int32)
        arr_a[:NGB, :] = oA.reshape(NGB, GB)
        arr_b = np.zeros((P, GB), np.int32)
        nbrow = (NSTAGE_B + GB - 1) // GB
        obp = np.zeros(nbrow * GB, np.int32)
        obp[:NSTAGE_B] = oB
        arr_b[:nbrow, :] = obp.reshape(nbrow, GB)
        arr_c = np.zeros((P, GB), np.int32)
        w = (oC[0::2].astype(np.int64)
             | (oC[1::2].astype(np.int64) << 16)).astype(np.int32)
        arr_c[1:NB + 1, :] = w.reshape(NB, GB)
        in_maps.append({"prims": prims, "primsT": primsT,
                        "offs_a": arr_a, "offs_b": arr_b, "offs_c": arr_c})

    import os
    trace_dir = os.environ.get("KERNEL_TRACE_DIR")
    res = run_bass_kernel_spmd(nc, in_maps, core_ids=list(range(NCORES)),
                               tmpdir=trace_dir)
    LAST_RESULTS = res

    out = np.empty((n, DIM, DIM), np.float32)
    for c in range(NCORES):
        o = np.asarray(res.results[c]["out"])  # (npos, P, 2, DIM) f16
        o = o.transpose(0, 2, 1, 3).reshape(npos, DIM, DIM).astype(np.float32)
        inv = np.empty(npos, np.int64)
        inv[orders[c]] = np.arange(npos)
        out[c * npos:(c + 1) * npos] = o[inv]

    ident = np.asarray(identity, dtype=np.float32)[0]
    if not np.allclose(ident, np.eye(DIM, dtype=np.float32)):
        out = np.einsum("ij,njk->nik", ident, out).astype(np.float32)
    return out
